# revision 1
# baseline (speedup 1.0000x reference)
"""BitLinear (BitNet b1.58) Trainium2 Bass kernel, token-sharded across 8 cores.

y = (round(clip(x/a_scale*127)) @ clip(round(W/w_scale),-1,1)^T) * w_scale*a_scale/127
  w_scale = mean(|W|)+eps (scalar), a_scale = max|x| per token + eps.

Strategy (per core, SPMD x8):
- x is sharded by tokens (16384/8 = 2048 per core); W replicated.
- Quantized activations (ints in [-127,127]) and ternary weights are exact in
  bf16; fp32 PSUM accumulation of <=2048 products (each |.|<=127) is exact, so
  the bf16 matmul is bit-exact integer arithmetic.
- Per token block [128, D]: abs-max reduce (DVE), round via the +-2^23 fp32
  trick (ACT+DVE), bf16 cast, SBUF->SBUF DMA-transpose into [D/128, 128, 128]
  i-major chunks, PE matmul vs pre-transposed ternary weights, scaled PSUM
  drain (ACT) with per-token output scale, DMA out.
- W: pass 1 reduces sum|W| (-> w_scale), pass 2 re-streams W, quantizes
  (clamp to +-1.4999999 then round trick) and DMA-transposes into a resident
  bf16 [128, D/128, O] rhs tensor.
"""

import sys

sys.path.insert(0, "/opt/trn_rl_repo")

import numpy as np

import concourse.bacc as bacc
import concourse.bass as bass
import concourse.bass_isa as bass_isa
import concourse.mybir as mybir
import concourse.tile as tile

P = 128
MM_N = 512  # free-dim per matmul (one PSUM bank)
EXP23 = 12582912.0  # 1.5*2**23; v + C - C == RNE round for |v| < 2**22
EPS = 1e-8
CLIP_HI = 1.4999999  # largest f32 < 1.5; clamp-then-round == clip(round(.),-1,1)
N_CORES = 8
F32 = mybir.dt.float32
BF16 = mybir.dt.bfloat16
ALU = mybir.AluOpType
AFT = mybir.ActivationFunctionType
AX = mybir.AxisListType


def emit_bitlinear(
    tc: "tile.TileContext",
    y: "bass.AP",
    xs: "bass.AP",
    w: "bass.AP",
    repeat: int = 1,
    variant: str = "full",
):
    nc = tc.nc
    T, D = xs.shape
    O = w.shape[0]
    TB, NI, NR = T // P, D // P, O // P
    NOB = O // MM_N
    no_wphase = variant in ("no_wphase", "mm1")
    mm1 = variant == "mm1"

    from contextlib import ExitStack

    for _rep in range(repeat):
        _emit_bitlinear_once(tc, y, xs, w, variant, _rep)


def _emit_bitlinear_once(
    tc: "tile.TileContext",
    y: "bass.AP",
    xs: "bass.AP",
    w: "bass.AP",
    variant: str,
    rep: int = 0,
):
    nc = tc.nc
    T, D = xs.shape
    O = w.shape[0]
    TB, NI, NR = T // P, D // P, O // P
    NOB = O // MM_N
    no_wphase = variant in ("no_wphase", "mm1")
    mm1 = variant == "mm1"
    NRES = NR - 3  # W tiles kept resident; the rest stream via the xe pool

    from contextlib import ExitStack

    with ExitStack() as ctx:
        small = ctx.enter_context(tc.tile_pool(name=f"small{rep}", bufs=1))
        wqt_pool = ctx.enter_context(tc.tile_pool(name=f"wqt{rep}", bufs=1))
        ps_pool = ctx.enter_context(
            tc.tile_pool(name=f"psp{rep}", bufs=8, space="PSUM")
        )
        sc_pool = ctx.enter_context(tc.tile_pool(name=f"scp{rep}", bufs=4))
        # early-chain pools live across the whole kernel but are small; the
        # early aqT tiles must outlive the W phase (PE reads them later)
        aqte_pool = ctx.enter_context(tc.tile_pool(name=f"aqte{rep}", bufs=2))

        wqT = wqt_pool.tile([P, NI, O], BF16)
        w_scale = small.tile([P, 1], F32)

        def x_chain(tb, xpool_, aq_pool_, aqt_pool_):
            xt = xpool_.tile([P, D], F32, tag="x", name=f"x_{tb}")
            nc.sync.dma_start(xt[:], xs[tb * P : (tb + 1) * P, :])
            amax = sc_pool.tile([P, 1], F32, tag="amax", name=f"amax_{tb}")
            nc.vector.tensor_reduce(
                amax[:], xt[:], axis=AX.X, op=ALU.max, apply_absolute_value=True
            )
            a_eps = sc_pool.tile([P, 1], F32, tag="aeps", name=f"aeps_{tb}",
                                 bufs=TB)
            nc.vector.tensor_scalar_add(a_eps[:], amax[:], EPS)
            rcp = sc_pool.tile([P, 1], F32, tag="rcp", name=f"rcp_{tb}")
            nc.vector.reciprocal(rcp[:], a_eps[:])
            inv127 = sc_pool.tile([P, 1], F32, tag="i127", name=f"i127_{tb}")
            nc.vector.tensor_scalar_mul(inv127[:], rcp[:], 127.0)
            # round in place: xt <- xt*inv127 + C (ACT), then -C -> bf16 (DVE)
            nc.scalar.activation(
                xt[:], xt[:], AFT.Copy, bias=EXP23, scale=inv127[:]
            )
            aq = aq_pool_.tile([P, D], BF16, tag="aq", name=f"aq_{tb}")
            nc.vector.tensor_scalar_add(aq[:], xt[:], -EXP23)
            # aqT[p, j, t] = aq[t, j*128 + p]
            aqT = aqt_pool_.tile([P, NI, P], BF16, tag="aqT", name=f"aqT_{tb}")
            nc.scalar.dma_start(aqT[:], aq[:], transpose=True)
            return aqT, a_eps

        early = {}

        # ---- W phase: single read; NRES tiles resident, 3 streamed through
        # the xe pool, which also hosts two early token chains.
        if no_wphase:
            nc.gpsimd.memset(wqT[:], 1.0)
            nc.gpsimd.memset(w_scale[:], 0.01)
        else:
            with tc.tile_pool(name=f"wres{rep}", bufs=NRES) as wres, \
                 tc.tile_pool(name=f"wqrp{rep}", bufs=2) as wqrp, \
                 tc.tile_pool(name=f"xe{rep}", bufs=2) as xe_pool, \
                 tc.tile_pool(name=f"aqe{rep}", bufs=1) as aqe_pool:
                wsums = small.tile([P, NR], F32)
                wts = []

                def w_load(r, pool, tag, keep, name_pfx="wt"):
                    wt = pool.tile([P, D], F32, tag=tag,
                                   name=f"{name_pfx}_{r}")
                    nc.sync.dma_start(wt[:], w[r * P : (r + 1) * P, :])
                    if keep:
                        wts.append(wt)
                        return wt
                    if r % 2 == 0:
                        nc.vector.tensor_reduce(
                            wsums[:, r : r + 1], wt[:], axis=AX.X, op=ALU.add,
                            apply_absolute_value=True,
                        )
                    else:
                        trash = wqrp.tile([P, D], BF16, tag="wqr",
                                          name=f"trash_{r}")
                        nc.scalar.activation(
                            trash[:], wt[:], AFT.Abs,
                            accum_out=wsums[:, r : r + 1],
                        )
                    wts.append(wt)
                    return wt

                for r in range(NRES):
                    w_load(r, wres, "wt", keep=False)
                # two early token chains through the xe/aqe pools; the last 3
                # W tiles are reduce-only here and re-read before quantize
                early[0] = x_chain(0, xe_pool, aqe_pool, aqte_pool)
                for r in range(NRES, NR):
                    wt = w_load(r, xe_pool, "x", keep=True, name_pfx="wta")
                    wts.pop()
                    if r % 2 == 0:
                        nc.vector.tensor_reduce(
                            wsums[:, r : r + 1], wt[:], axis=AX.X, op=ALU.add,
                            apply_absolute_value=True,
                        )
                    else:
                        trash = wqrp.tile([P, D], BF16, tag="wqr",
                                          name=f"trash_{r}")
                        nc.scalar.activation(
                            trash[:], wt[:], AFT.Abs,
                            accum_out=wsums[:, r : r + 1],
                        )
                    if r == NRES:
                        early[1] = x_chain(1, xe_pool, aqe_pool, aqte_pool)

                wsum1 = small.tile([P, 1], F32)
                nc.vector.tensor_reduce(
                    wsum1[:], wsums[:], axis=AX.X, op=ALU.add
                )
                wsum_all = small.tile([P, 1], F32)
                nc.gpsimd.partition_all_reduce(
                    wsum_all[:], wsum1[:], channels=P,
                    reduce_op=bass_isa.ReduceOp.add,
                )
                nc.vector.tensor_scalar(
                    w_scale[:], wsum_all[:], 1.0 / (O * D), EPS, op0=ALU.mult,
                    op1=ALU.add,
                )
                inv_w = small.tile([P, 1], F32)
                nc.vector.reciprocal(inv_w[:], w_scale[:])

                # re-read the streamed tiles now that w_scale is known
                for r in range(NRES, NR):
                    w_load(r, xe_pool, "x", keep=True, name_pfx="wtb")

                # in-place ternary quantize (DVE/GPSIMD alternate); bf16 via
                # ACT (even) / DVE (odd); transpose into wqT from SP
                for r in range(NR):
                    u = wts[r]
                    eng = nc.vector if r % 2 == 0 else nc.gpsimd
                    eng.tensor_scalar(
                        u[:], u[:], inv_w[:], CLIP_HI, op0=ALU.mult,
                        op1=ALU.min,
                    )
                    eng.tensor_scalar(
                        u[:], u[:], -CLIP_HI, EXP23, op0=ALU.max, op1=ALU.add
                    )
                    wqr = wqrp.tile([P, D], BF16, tag="wqr", name=f"wqr_{r}")
                    if r % 2 == 0:
                        nc.scalar.activation(
                            wqr[:], u[:], AFT.Copy, bias=-EXP23
                        )
                    else:
                        nc.vector.tensor_scalar_add(wqr[:], u[:], -EXP23)
                    nc.sync.dma_start(
                        wqT[:, :, r * P : (r + 1) * P], wqr[:], transpose=True
                    )

        # ---- token pipeline pools (reuse the released W-phase SBUF)
        xpool = ctx.enter_context(tc.tile_pool(name=f"xp{rep}", bufs=6))
        aq_pool = ctx.enter_context(tc.tile_pool(name=f"aqp{rep}", bufs=4))
        aqt_pool = ctx.enter_context(tc.tile_pool(name=f"aqtp{rep}", bufs=8))
        ypool = ctx.enter_context(tc.tile_pool(name=f"yp{rep}", bufs=6))

        es_cache = {}

        def mm_quarter(tb, ob, aqT, a_eps):
            if tb not in es_cache:
                es = sc_pool.tile([P, 1], F32, tag="es", name=f"es_{tb}",
                                  bufs=8)
                nc.vector.tensor_scalar(
                    es[:], a_eps[:], w_scale[:], 1.0 / 127.0, op0=ALU.mult,
                    op1=ALU.mult,
                )
                es_cache[tb] = es
            es = es_cache[tb]
            ps = ps_pool.tile([P, MM_N], F32, tag="ps", name=f"ps_{tb}_{ob}")
            NJ = 1 if mm1 else NI
            for j in range(NJ):
                nc.tensor.matmul(
                    ps[:],
                    lhsT=aqT[:, j, :],
                    rhs=wqT[:, j, ob * MM_N : (ob + 1) * MM_N],
                    start=(j == 0),
                    stop=(j == NJ - 1),
                )
            ysb = ypool.tile([P, MM_N], F32, tag="y", name=f"y_{tb}_{ob}")
            nc.scalar.activation(ysb[:], ps[:], AFT.Copy, scale=es[:])
            nc.gpsimd.dma_start(
                y[tb * P : (tb + 1) * P, ob * MM_N : (ob + 1) * MM_N], ysb[:]
            )

        # ---- main loop
        for tb in range(TB):
            if tb in early:
                aqT, a_eps = early[tb]
            else:
                aqT, a_eps = x_chain(tb, xpool, aq_pool, aqt_pool)
            for ob in range(NOB):
                mm_quarter(tb, ob, aqT, a_eps)


_NC_CACHE: dict = {}


def _get_nc(
    T: int, D: int, O: int, repeat: int = 1, variant: str = "full"
) -> "bass.Bass":
    key = (T, D, O, repeat, variant)
    if key not in _NC_CACHE:
        nc = bacc.Bacc("TRN2", target_bir_lowering=False, debug=False)
        xs = nc.dram_tensor("xs", [T, D], F32, kind="ExternalInput").ap()
        w = nc.dram_tensor("w", [O, D], F32, kind="ExternalInput").ap()
        y = nc.dram_tensor("y", [T, O], F32, kind="ExternalOutput").ap()
        with tile.TileContext(nc) as tc:
            emit_bitlinear(tc, y, xs, w, repeat=repeat, variant=variant)
        nc.compile()
        _NC_CACHE[key] = nc
    return _NC_CACHE[key]


def kernel(
    x: np.ndarray, weight: np.ndarray, _trace: bool = False, _repeat: int = 1
):
    from concourse.bass_utils import run_bass_kernel_spmd

    x = np.asarray(x, dtype=np.float32)
    weight = np.ascontiguousarray(np.asarray(weight, dtype=np.float32))
    B, S, D = x.shape
    O = weight.shape[0]
    tokens = B * S
    Tc = tokens // N_CORES
    xf = np.ascontiguousarray(x.reshape(tokens, D))

    nc = _get_nc(Tc, D, O, repeat=_repeat)
    in_maps = [
        {"xs": np.ascontiguousarray(xf[c * Tc : (c + 1) * Tc]), "w": weight}
        for c in range(N_CORES)
    ]
    res = run_bass_kernel_spmd(
        nc, in_maps, list(range(N_CORES)), trace=_trace
    )
    out = np.concatenate([res.results[c]["y"] for c in range(N_CORES)], axis=0)
    out = out.reshape(B, S, O)
    if _trace:
        return out, res
    return out



# revision 17
# speedup vs baseline: 1.1899x; 1.1899x over previous
"""BitLinear (BitNet b1.58) Trainium2 Bass kernel, token-sharded across 8 cores.

y = (round(clip(x/a_scale*127)) @ clip(round(W/w_scale),-1,1)^T) * w_scale*a_scale/127
  w_scale = mean(|W|)+eps (scalar), a_scale = max|x| per token + eps.

Strategy (per core, SPMD x8):
- x is sharded by tokens (16384/8 = 2048 per core); W replicated.
- Quantized activations (ints in [-127,127]) and ternary weights are exact in
  bf16; fp32 PSUM accumulation of <=2048 products (each |.|<=127) is exact, so
  the bf16 matmul is bit-exact integer arithmetic.
- Per token block [128, D]: abs-max reduce (DVE), round via the +-2^23 fp32
  trick (ACT+DVE), bf16 cast, SBUF->SBUF DMA-transpose into [D/128, 128, 128]
  i-major chunks, PE matmul vs pre-transposed ternary weights, scaled PSUM
  drain (ACT) with per-token output scale, DMA out.
- W: pass 1 reduces sum|W| (-> w_scale), pass 2 re-streams W, quantizes
  (clamp to +-1.4999999 then round trick) and DMA-transposes into a resident
  bf16 [128, D/128, O] rhs tensor.
"""

import sys

sys.path.insert(0, "/opt/trn_rl_repo")

import numpy as np

import concourse.bacc as bacc
import concourse.bass as bass
import concourse.bass_isa as bass_isa
import concourse.mybir as mybir
import concourse.tile as tile

P = 128
MM_N = 512  # free-dim per matmul (one PSUM bank)
EXP23 = 12582912.0  # 1.5*2**23; v + C - C == RNE round for |v| < 2**22
EPS = 1e-8
CLIP_HI = 1.4999999  # largest f32 < 1.5; clamp-then-round == clip(round(.),-1,1)
N_CORES = 8
F32 = mybir.dt.float32
BF16 = mybir.dt.bfloat16
ALU = mybir.AluOpType
AFT = mybir.ActivationFunctionType
AX = mybir.AxisListType


def emit_bitlinear(
    tc: "tile.TileContext",
    y: "bass.AP",
    xs: "bass.AP",
    w: "bass.AP",
    repeat: int = 1,
    variant: str = "full",
):
    nc = tc.nc
    T, D = xs.shape
    O = w.shape[0]
    TB, NI, NR = T // P, D // P, O // P
    NOB = O // MM_N
    no_wphase = variant in ("no_wphase", "mm1")
    mm1 = variant == "mm1"

    from contextlib import ExitStack

    if variant == "pe_probe":
        _emit_pe_probe(tc, y, xs, w, repeat)
        return
    for _rep in range(repeat):
        if variant in ("v2", "v2p"):
            _emit_bitlinear_v2(tc, y, xs, w, variant, _rep)
        else:
            _emit_bitlinear_once(tc, y, xs, w, variant, _rep)


def _emit_bitlinear_once(
    tc: "tile.TileContext",
    y: "bass.AP",
    xs: "bass.AP",
    w: "bass.AP",
    variant: str,
    rep: int = 0,
):
    nc = tc.nc
    T, D = xs.shape
    O = w.shape[0]
    TB, NI, NR = T // P, D // P, O // P
    NOB = O // MM_N
    mm_only = variant == "mm_only"
    noxt = variant == "noxt"
    no_wphase = variant in ("no_wphase", "mm1", "mm_only", "noxt")
    mm1 = variant == "mm1"
    NRES = NR - 3  # W tiles kept resident; the rest stream via the xe pool

    from contextlib import ExitStack

    with ExitStack() as ctx:
        small = ctx.enter_context(tc.tile_pool(name=f"small{rep}", bufs=1))
        wqt_pool = ctx.enter_context(tc.tile_pool(name=f"wqt{rep}", bufs=1))
        ps_pool = ctx.enter_context(
            tc.tile_pool(name=f"psp{rep}", bufs=8, space="PSUM")
        )
        sc_pool = ctx.enter_context(tc.tile_pool(name=f"scp{rep}", bufs=4))
        # early-chain pools live across the whole kernel but are small; the
        # early aqT tiles must outlive the W phase (PE reads them later)
        aqte_pool = ctx.enter_context(tc.tile_pool(name=f"aqte{rep}", bufs=2))

        wqT = wqt_pool.tile([P, NI, O], BF16)
        w_scale = small.tile([P, 1], F32)

        def x_chain(tb, xpool_, aq_pool_, aqt_pool_):
            xt = xpool_.tile([P, D], F32, tag="x", name=f"x_{tb}")
            nc.sync.dma_start(xt[:], xs[tb * P : (tb + 1) * P, :])
            amax = sc_pool.tile([P, 1], F32, tag="amax", name=f"amax_{tb}")
            nc.vector.tensor_reduce(
                amax[:], xt[:], axis=AX.X, op=ALU.max, apply_absolute_value=True
            )
            a_eps = sc_pool.tile([P, 1], F32, tag="aeps", name=f"aeps_{tb}",
                                 bufs=TB)
            nc.vector.tensor_scalar_add(a_eps[:], amax[:], EPS)
            rcp = sc_pool.tile([P, 1], F32, tag="rcp", name=f"rcp_{tb}")
            nc.vector.reciprocal(rcp[:], a_eps[:])
            inv127 = sc_pool.tile([P, 1], F32, tag="i127", name=f"i127_{tb}")
            nc.vector.tensor_scalar_mul(inv127[:], rcp[:], 127.0)
            # round in place: xt <- xt*inv127 + C (ACT), then -C -> bf16 (DVE)
            nc.scalar.activation(
                xt[:], xt[:], AFT.Copy, bias=EXP23, scale=inv127[:]
            )
            aq = aq_pool_.tile([P, D], BF16, tag="aq", name=f"aq_{tb}")
            nc.vector.tensor_scalar_add(aq[:], xt[:], -EXP23)
            # aqT[p, j, t] = aq[t, j*128 + p]
            aqT = aqt_pool_.tile([P, NI, P], BF16, tag="aqT", name=f"aqT_{tb}")
            if noxt:
                # timing probe: keep the aq->aqT dependency but skip the
                # expensive transpose (aqT holds garbage; output unused)
                nc.vector.tensor_copy(aqT[:, 0, :], aq[:, 0:P])
            else:
                nc.scalar.dma_start(aqT[:], aq[:], transpose=True)
            return aqT, a_eps

        early = {}

        # ---- W phase: single read; NRES tiles resident, 3 streamed through
        # the xe pool, which also hosts two early token chains.
        if no_wphase:
            nc.gpsimd.memset(wqT[:], 1.0)
            nc.gpsimd.memset(w_scale[:], 0.01)
        else:
            with tc.tile_pool(name=f"wres{rep}", bufs=NRES) as wres, \
                 tc.tile_pool(name=f"wqrp{rep}", bufs=2) as wqrp, \
                 tc.tile_pool(name=f"xe{rep}", bufs=2) as xe_pool, \
                 tc.tile_pool(name=f"aqe{rep}", bufs=1) as aqe_pool:
                wsums = small.tile([P, NR], F32)
                wts = []

                def w_load(r, pool, tag, keep, name_pfx="wt"):
                    wt = pool.tile([P, D], F32, tag=tag,
                                   name=f"{name_pfx}_{r}")
                    nc.sync.dma_start(wt[:], w[r * P : (r + 1) * P, :])
                    if keep:
                        wts.append(wt)
                        return wt
                    if r % 2 == 0:
                        nc.vector.tensor_reduce(
                            wsums[:, r : r + 1], wt[:], axis=AX.X, op=ALU.add,
                            apply_absolute_value=True,
                        )
                    else:
                        trash = wqrp.tile([P, D], BF16, tag="wqr",
                                          name=f"trash_{r}")
                        nc.scalar.activation(
                            trash[:], wt[:], AFT.Abs,
                            accum_out=wsums[:, r : r + 1],
                        )
                    wts.append(wt)
                    return wt

                for r in range(NRES):
                    w_load(r, wres, "wt", keep=False)
                # two early token chains through the xe/aqe pools; the last 3
                # W tiles are reduce-only here and re-read before quantize
                early[0] = x_chain(0, xe_pool, aqe_pool, aqte_pool)
                for r in range(NRES, NR):
                    wt = w_load(r, xe_pool, "x", keep=True, name_pfx="wta")
                    wts.pop()
                    if r % 2 == 0:
                        nc.vector.tensor_reduce(
                            wsums[:, r : r + 1], wt[:], axis=AX.X, op=ALU.add,
                            apply_absolute_value=True,
                        )
                    else:
                        trash = wqrp.tile([P, D], BF16, tag="wqr",
                                          name=f"trash_{r}")
                        nc.scalar.activation(
                            trash[:], wt[:], AFT.Abs,
                            accum_out=wsums[:, r : r + 1],
                        )
                    if r == NRES:
                        early[1] = x_chain(1, xe_pool, aqe_pool, aqte_pool)

                wsum1 = small.tile([P, 1], F32)
                nc.vector.tensor_reduce(
                    wsum1[:], wsums[:], axis=AX.X, op=ALU.add
                )
                # cross-partition all-reduce via PE: ones^T @ wsum1 puts
                # sum_p wsum1[p] on every output partition (f32, exact
                # enough; the GPSIMD partition_all_reduce ucode op costs
                # ~0.5 ms on HW and sits on the w_scale critical path)
                ones = small.tile([P, P], F32)
                nc.vector.memset(ones[:], 1.0)
                ps_ws = ps_pool.tile([P, MM_N], F32, tag="ps", name="ps_ws")
                nc.tensor.matmul(
                    ps_ws[:, 0:1], lhsT=ones[:], rhs=wsum1[:], start=True,
                    stop=True,
                )
                nc.scalar.activation(
                    w_scale[:], ps_ws[:, 0:1], AFT.Copy, bias=EPS,
                    scale=1.0 / (O * D),
                )
                inv_w = small.tile([P, 1], F32)
                nc.vector.reciprocal(inv_w[:], w_scale[:])

                # re-read the streamed tiles now that w_scale is known
                for r in range(NRES, NR):
                    w_load(r, xe_pool, "x", keep=True, name_pfx="wtb")

                # in-place ternary quantize (DVE/GPSIMD alternate); bf16 via
                # ACT (even) / DVE (odd); transpose into wqT from SP
                for r in range(NR):
                    u = wts[r]
                    eng = nc.vector if r % 2 == 0 else nc.gpsimd
                    eng.tensor_scalar(
                        u[:], u[:], inv_w[:], CLIP_HI, op0=ALU.mult,
                        op1=ALU.min,
                    )
                    eng.tensor_scalar(
                        u[:], u[:], -CLIP_HI, EXP23, op0=ALU.max, op1=ALU.add
                    )
                    wqr = wqrp.tile([P, D], BF16, tag="wqr", name=f"wqr_{r}")
                    if r % 2 == 0:
                        nc.scalar.activation(
                            wqr[:], u[:], AFT.Copy, bias=-EXP23
                        )
                    else:
                        nc.vector.tensor_scalar_add(wqr[:], u[:], -EXP23)
                    nc.sync.dma_start(
                        wqT[:, :, r * P : (r + 1) * P], wqr[:], transpose=True
                    )

        # ---- token pipeline pools (reuse the released W-phase SBUF)
        xpool = ctx.enter_context(tc.tile_pool(name=f"xp{rep}", bufs=6))
        aq_pool = ctx.enter_context(tc.tile_pool(name=f"aqp{rep}", bufs=4))
        aqt_pool = ctx.enter_context(tc.tile_pool(name=f"aqtp{rep}", bufs=8))
        ypool = ctx.enter_context(tc.tile_pool(name=f"yp{rep}", bufs=6))

        es_cache = {}

        def mm_quarter(tb, ob, aqT, a_eps):
            if tb not in es_cache:
                es = sc_pool.tile([P, 1], F32, tag="es", name=f"es_{tb}",
                                  bufs=8)
                nc.vector.tensor_scalar(
                    es[:], a_eps[:], w_scale[:], 1.0 / 127.0, op0=ALU.mult,
                    op1=ALU.mult,
                )
                es_cache[tb] = es
            es = es_cache[tb]
            ps = ps_pool.tile([P, MM_N], F32, tag="ps", name=f"ps_{tb}_{ob}")
            NJ = 1 if mm1 else NI
            for j in range(NJ):
                nc.tensor.matmul(
                    ps[:],
                    lhsT=aqT[:, j, :],
                    rhs=wqT[:, j, ob * MM_N : (ob + 1) * MM_N],
                    start=(j == 0),
                    stop=(j == NJ - 1),
                )
            ysb = ypool.tile([P, MM_N], F32, tag="y", name=f"y_{tb}_{ob}")
            nc.scalar.activation(ysb[:], ps[:], AFT.Copy, scale=es[:])
            nc.gpsimd.dma_start(
                y[tb * P : (tb + 1) * P, ob * MM_N : (ob + 1) * MM_N], ysb[:]
            )

        # ---- main loop
        if mm_only:
            aqT0 = aqt_pool.tile([P, NI, P], BF16, tag="aqT", name="aqT_st")
            nc.gpsimd.memset(aqT0[:], 1.0)
            aeps0 = sc_pool.tile([P, 1], F32, tag="aeps", name="aeps_st",
                                 bufs=TB)
            nc.gpsimd.memset(aeps0[:], 1.0)
        for tb in range(TB):
            if mm_only:
                aqT, a_eps = aqT0, aeps0
            elif tb in early:
                aqT, a_eps = early[tb]
            else:
                aqT, a_eps = x_chain(tb, xpool, aq_pool, aqt_pool)
            for ob in range(NOB):
                mm_quarter(tb, ob, aqT, a_eps)


def _emit_pe_probe(tc, y, xs, w, repeat):
    """Pure-PE timing probe: per rep, 1024 N=512 bf16 matmuls (16 LDW-chains
    of 16 accumulating MMs x 4 banks), tiny ACT drains so nothing is DCE'd.
    No DMA, no x/W processing."""
    nc = tc.nc
    T, D = xs.shape
    O = w.shape[0]
    TB, NI = T // P, D // P
    NOB = O // MM_N
    with tc.tile_pool(name="pp_s", bufs=1) as small, \
         tc.tile_pool(name="pp_ps", bufs=8, space="PSUM") as ps_pool, \
         tc.tile_pool(name="pp_y", bufs=4) as ypool:
        aqT = small.tile([P, NI, P], BF16)
        wqT = small.tile([P, NI, MM_N], BF16)
        nc.gpsimd.memset(aqT[:], 1.0)
        nc.gpsimd.memset(wqT[:], 0.5)
        for rep in range(repeat):
            ysb = ypool.tile([P, TB * NOB], F32, tag="y", name=f"yp_{rep}")
            for tb in range(TB):
                for ob in range(NOB):
                    ps = ps_pool.tile([P, MM_N], F32, tag="ps",
                                      name=f"ps_{rep}_{tb}_{ob}")
                    for j in range(NI):
                        nc.tensor.matmul(
                            ps[:], lhsT=aqT[:, j, :], rhs=wqT[:, j, :],
                            start=(j == 0), stop=(j == NI - 1),
                        )
                    col = tb * NOB + ob
                    nc.scalar.activation(
                        ysb[:, col : col + 1], ps[:, 0:1], AFT.Copy
                    )
            nc.gpsimd.dma_start(y[0:P, rep * 64 : rep * 64 + 64], ysb[:])


def _emit_bitlinear_v2(
    tc: "tile.TileContext",
    y: "bass.AP",
    xs: "bass.AP",
    w: "bass.AP",
    variant: str,
    rep: int = 0,
):
    """Restructured emission: W phase streams twice (no residency), wqT is
    4 per-ob tiles so matmuls gate on exactly the slice they read, and DMA
    traffic is split across the three rings (SP: W loads, ACT: transposes,
    Pool/SWDGE: x loads + y stores).

    variant v2: token x-chains coupled to the MM sweep (baseline style).
    variant v2p: all 16 x-chains emitted up front (aqT fully staged).
    """
    nc = tc.nc
    T, D = xs.shape
    O = w.shape[0]
    TB, NI, NR = T // P, D // P, O // P
    NOB = O // MM_N
    RPO = NR // NOB  # W row-tiles per ob slice (4)
    prechain = variant == "v2p"

    from contextlib import ExitStack

    with ExitStack() as ctx:
        small = ctx.enter_context(tc.tile_pool(name=f"small{rep}", bufs=1))
        wqt_pool = ctx.enter_context(tc.tile_pool(name=f"wqt{rep}", bufs=NOB))
        ps_pool = ctx.enter_context(
            tc.tile_pool(name=f"psp{rep}", bufs=8, space="PSUM")
        )
        sc_pool = ctx.enter_context(tc.tile_pool(name=f"scp{rep}", bufs=4))
        xpool = ctx.enter_context(tc.tile_pool(name=f"xp{rep}", bufs=3))
        aq_pool = ctx.enter_context(tc.tile_pool(name=f"aqp{rep}", bufs=2))
        aqt_pool = ctx.enter_context(
            tc.tile_pool(name=f"aqtp{rep}", bufs=TB if prechain else 8)
        )
        ypool = ctx.enter_context(tc.tile_pool(name=f"yp{rep}", bufs=2))

        wqT = [
            wqt_pool.tile([P, NI, MM_N], BF16, tag="wqT", name=f"wqT_{ob}")
            for ob in range(NOB)
        ]
        w_scale = small.tile([P, 1], F32)
        inv_w = small.tile([P, 1], F32)
        wsums = small.tile([P, NR], F32)

        def x_chain(tb):
            xt = xpool.tile([P, D], F32, tag="x", name=f"x_{tb}")
            nc.gpsimd.dma_start(xt[:], xs[tb * P : (tb + 1) * P, :])
            amax = sc_pool.tile([P, 1], F32, tag="amax", name=f"amax_{tb}")
            nc.vector.tensor_reduce(
                amax[:], xt[:], axis=AX.X, op=ALU.max, apply_absolute_value=True
            )
            a_eps = sc_pool.tile([P, 1], F32, tag="aeps", name=f"aeps_{tb}",
                                 bufs=TB)
            nc.vector.tensor_scalar_add(a_eps[:], amax[:], EPS)
            rcp = sc_pool.tile([P, 1], F32, tag="rcp", name=f"rcp_{tb}")
            nc.vector.reciprocal(rcp[:], a_eps[:])
            inv127 = sc_pool.tile([P, 1], F32, tag="i127", name=f"i127_{tb}")
            nc.vector.tensor_scalar_mul(inv127[:], rcp[:], 127.0)
            # round in place: xt <- xt*inv127 + C (ACT), then -C -> bf16 (DVE)
            nc.scalar.activation(
                xt[:], xt[:], AFT.Copy, bias=EXP23, scale=inv127[:]
            )
            aq = aq_pool.tile([P, D], BF16, tag="aq", name=f"aq_{tb}")
            nc.vector.tensor_scalar_add(aq[:], xt[:], -EXP23)
            aqT = aqt_pool.tile([P, NI, P], BF16, tag="aqT", name=f"aqT_{tb}")
            nc.scalar.dma_start(aqT[:], aq[:], transpose=True)
            return aqT, a_eps

        # ---- W pass 1: stream + abs-reduce (DVE even / ACT-accum odd);
        # first x-chains interleave (x loads ride the Pool/SWDGE ring, aq
        # transposes the ACT ring — neither contends with pass-1 SP loads)
        n_early = TB if prechain else 3
        chains = {}
        with tc.tile_pool(name=f"ws{rep}", bufs=3) as wstream, \
             tc.tile_pool(name=f"wqr{rep}", bufs=2) as wqrp:
            for r in range(NR):
                wt = wstream.tile([P, D], F32, tag="wt", name=f"wta_{r}")
                nc.sync.dma_start(wt[:], w[r * P : (r + 1) * P, :])
                if r % 2 == 0:
                    nc.vector.tensor_reduce(
                        wsums[:, r : r + 1], wt[:], axis=AX.X, op=ALU.add,
                        apply_absolute_value=True,
                    )
                else:
                    trash = wqrp.tile([P, D], BF16, tag="wqr",
                                      name=f"trash_{r}")
                    nc.scalar.activation(
                        trash[:], wt[:], AFT.Abs,
                        accum_out=wsums[:, r : r + 1],
                    )
                if r % 5 == 1 and len(chains) < 3:
                    chains[len(chains)] = x_chain(len(chains))
            wsum1 = small.tile([P, 1], F32)
            nc.vector.tensor_reduce(wsum1[:], wsums[:], axis=AX.X, op=ALU.add)
            # cross-partition all-reduce + broadcast via PE (see v1 note)
            ones = small.tile([P, P], F32)
            nc.vector.memset(ones[:], 1.0)
            ps_ws = ps_pool.tile([P, MM_N], F32, tag="ps", name="ps_ws")
            nc.tensor.matmul(
                ps_ws[:, 0:1], lhsT=ones[:], rhs=wsum1[:], start=True,
                stop=True,
            )
            nc.scalar.activation(
                w_scale[:], ps_ws[:, 0:1], AFT.Copy, bias=EPS,
                scale=1.0 / (O * D),
            )
            nc.vector.reciprocal(inv_w[:], w_scale[:])

            # ---- W pass 2: stream + quantize + cast + transpose (wq
            # transposes ride the SP ring with the pass-2 loads; the ACT
            # ring is reserved for aq transposes)
            for r in range(NR):
                wt = wstream.tile([P, D], F32, tag="wt", name=f"wtb_{r}")
                nc.sync.dma_start(wt[:], w[r * P : (r + 1) * P, :])
                eng = nc.vector if r % 2 == 0 else nc.gpsimd
                eng.tensor_scalar(
                    wt[:], wt[:], inv_w[:], CLIP_HI, op0=ALU.mult, op1=ALU.min
                )
                eng.tensor_scalar(
                    wt[:], wt[:], -CLIP_HI, EXP23, op0=ALU.max, op1=ALU.add
                )
                wqr = wqrp.tile([P, D], BF16, tag="wqr", name=f"wqr_{r}")
                if r % 2 == 0:
                    nc.scalar.activation(wqr[:], wt[:], AFT.Copy, bias=-EXP23)
                else:
                    nc.vector.tensor_scalar_add(wqr[:], wt[:], -EXP23)
                ob, off = r // RPO, (r % RPO) * P
                nc.sync.dma_start(
                    wqT[ob][:, :, off : off + P], wqr[:], transpose=True
                )
                if 3 <= len(chains) < n_early:
                    chains[len(chains)] = x_chain(len(chains))

        es_cache = {}

        def mm_quarter(tb, ob, aqT, a_eps):
            if tb not in es_cache:
                es = sc_pool.tile([P, 1], F32, tag="es", name=f"es_{tb}",
                                  bufs=8)
                nc.vector.tensor_scalar(
                    es[:], a_eps[:], w_scale[:], 1.0 / 127.0, op0=ALU.mult,
                    op1=ALU.mult,
                )
                es_cache[tb] = es
            es = es_cache[tb]
            ps = ps_pool.tile([P, MM_N], F32, tag="ps", name=f"ps_{tb}_{ob}")
            for j in range(NI):
                nc.tensor.matmul(
                    ps[:],
                    lhsT=aqT[:, j, :],
                    rhs=wqT[ob][:, j, :],
                    start=(j == 0),
                    stop=(j == NI - 1),
                )
            return ps, es

        # ---- main loop
        for tb in range(TB):
            if tb in chains:
                aqT, a_eps = chains[tb]
            else:
                aqT, a_eps = x_chain(tb)
            ysb = ypool.tile([P, O], F32, tag="y", name=f"y_{tb}")
            for ob in range(NOB):
                ps, es = mm_quarter(tb, ob, aqT, a_eps)
                nc.scalar.activation(
                    ysb[:, ob * MM_N : (ob + 1) * MM_N], ps[:], AFT.Copy,
                    scale=es[:],
                )
            nc.sync.dma_start(y[tb * P : (tb + 1) * P, :], ysb[:])


_NC_CACHE: dict = {}


def _get_nc(
    T: int, D: int, O: int, repeat: int = 1, variant: str = "full"
) -> "bass.Bass":
    key = (T, D, O, repeat, variant)
    if key not in _NC_CACHE:
        nc = bacc.Bacc("TRN2", target_bir_lowering=False, debug=False)
        xs = nc.dram_tensor("xs", [T, D], F32, kind="ExternalInput").ap()
        w = nc.dram_tensor("w", [O, D], F32, kind="ExternalInput").ap()
        y = nc.dram_tensor("y", [T, O], F32, kind="ExternalOutput").ap()
        with tile.TileContext(nc) as tc:
            emit_bitlinear(tc, y, xs, w, repeat=repeat, variant=variant)
        nc.compile()
        _NC_CACHE[key] = nc
    return _NC_CACHE[key]


def kernel(
    x: np.ndarray, weight: np.ndarray, _trace: bool = False, _repeat: int = 1
):
    from concourse.bass_utils import run_bass_kernel_spmd

    x = np.asarray(x, dtype=np.float32)
    weight = np.ascontiguousarray(np.asarray(weight, dtype=np.float32))
    B, S, D = x.shape
    O = weight.shape[0]
    tokens = B * S
    Tc = tokens // N_CORES
    xf = np.ascontiguousarray(x.reshape(tokens, D))

    nc = _get_nc(Tc, D, O, repeat=_repeat)
    in_maps = [
        {"xs": np.ascontiguousarray(xf[c * Tc : (c + 1) * Tc]), "w": weight}
        for c in range(N_CORES)
    ]
    res = run_bass_kernel_spmd(
        nc, in_maps, list(range(N_CORES)), trace=_trace
    )
    out = np.concatenate([res.results[c]["y"] for c in range(N_CORES)], axis=0)
    out = out.reshape(B, S, O)
    if _trace:
        return out, res
    return out

